# revision 20
# baseline (speedup 1.0000x reference)
"""DeepSeekMoE Trainium2 kernel (8 NeuronCores, SPMD, expert-parallel).

Strategy:
  - Host computes top-2 routing AND the gate values (it needs softmax probs
    for the routing decision anyway).  Gate x (1-alpha) is applied on the
    host during the scatter-add of per-pair outputs, so the device computes
    UNGATED expert FFNs only -- no gate matmul/softmax machinery on device.
  - Expert parallel: core c holds routed expert c's full weights [D,F] and
    processes only the tokens routed to expert c (padded to a uniform
    capacity CAP so all cores run the same SPMD program).  Its yr output is
    final for those pairs (no cross-core reduction of routed outputs).
  - Shared experts are sharded 1:1: core c runs shared expert c over all
    T tokens; the host sums the 8 contributions (alpha/NS folded into w2_s).
    Shared chunks are interleaved between routed f-tiles to cover DMA ramps
    and the mm2 output tail.
  - All matmuls bf16 into fp32 PSUM.  Weights/activations are pre-packed on
    the host into [128, a, f] tile layouts so every DMA line is contiguous.
"""

import numpy as np
import ml_dtypes

import concourse.bacc as bacc
import concourse.tile as tile
import concourse.mybir as mybir
from concourse.bass_utils import run_bass_kernel_spmd

BF16 = ml_dtypes.bfloat16

B, S, D, F, E, NS, K = 2, 1024, 1024, 4096, 8, 8, 2
T = B * S
FS = F // NS            # shared expert hidden = 512
ALPHA = 0.5
N_CORES = 8
NFT = F // 128          # 32 f-tiles of the routed expert
NDT = D // 128          # 8 d-tiles

_NC = {}            # cap -> compiled program
LAST_RESULT = None  # BassKernelResults of the most recent run (for profiling)


def _build_program(cap):
    bf = mybir.dt.bfloat16
    f32 = mybir.dt.float32
    Act = mybir.ActivationFunctionType
    # two equal chunks: both big enough to hide LDWEIGHTS latency, and
    # each stays within one PSUM bank at the 0/512 tile offsets.
    # triples: (psum column offset, data column offset, width)
    half = cap // 2
    if cap > 512:
        chunks = [(0, 0, half), (512, half, cap - half)]
    else:
        chunks = [(0, 0, cap)]

    nc = bacc.Bacc("TRN2", target_bir_lowering=False, debug=False,
                   num_devices=N_CORES)

    # all inputs pre-packed host-side into [128, a, f] tile layout
    xp = nc.dram_tensor("xp", [128, NDT, cap], bf, kind="ExternalInput").ap()
    xf = nc.dram_tensor("xf", [128, NDT, T], bf, kind="ExternalInput").ap()
    w1 = nc.dram_tensor("w1", [NFT, 128, NDT, 128], bf,
                        kind="ExternalInput").ap()
    w3 = nc.dram_tensor("w3", [NFT, 128, NDT, 128], bf,
                        kind="ExternalInput").ap()
    w2 = nc.dram_tensor("w2", [NDT, 128, NFT, 128], bf,
                        kind="ExternalInput").ap()
    w1s = nc.dram_tensor("w1s", [128, NDT, FS], bf, kind="ExternalInput").ap()
    w3s = nc.dram_tensor("w3s", [128, NDT, FS], bf, kind="ExternalInput").ap()
    w2s = nc.dram_tensor("w2s", [128, FS // 128, D], bf,
                         kind="ExternalInput").ap()
    yr = nc.dram_tensor("yr", [128, NDT, cap], bf, kind="ExternalOutput").ap()
    ys = nc.dram_tensor("ys", [128, NDT, T], bf, kind="ExternalOutput").ap()

    with tile.TileContext(nc) as tc:
        with tc.tile_pool(name="const", bufs=1) as const, \
             tc.tile_pool(name="wst", bufs=8) as wst, \
             tc.tile_pool(name="w2st", bufs=4) as w2st, \
             tc.tile_pool(name="acts", bufs=1) as acts, \
             tc.tile_pool(name="xfp", bufs=2) as xfp, \
             tc.tile_pool(name="htp", bufs=1) as htp, \
             tc.tile_pool(name="hts", bufs=2) as hts, \
             tc.tile_pool(name="spool", bufs=3) as spool, \
             tc.tile_pool(name="outs", bufs=2) as outs, \
             tc.tile_pool(name="psum", bufs=4, space="PSUM") as psum:

            state = {}

            def load_w13(ft):
                W1 = wst.tile([128, NDT, 128], bf, tag="w1", name=f"w1_{ft}")
                nc.sync.dma_start(out=W1, in_=w1[ft])
                W3 = wst.tile([128, NDT, 128], bf, tag="w3", name=f"w3_{ft}")
                nc.sync.dma_start(out=W3, in_=w3[ft])
                state[("W13", ft)] = (W1, W3)

            def load_w2(dt):
                W2 = w2st.tile([128, NFT, 128], bf, tag="w2", name=f"w2_{dt}")
                nc.sync.dma_start(out=W2, in_=w2[dt])
                state[("W2", dt)] = W2

            def load_xf(ch):
                XF = xfp.tile([128, NDT, 512], bf, tag="xf", name=f"xf{ch}")
                nc.sync.dma_start(out=XF, in_=xf[:, :, ch * 512:(ch + 1) * 512])
                state[("XF", ch)] = XF

            def ffn13(ft):
                """h[ft] = silu(w1[ft]^T xp) * (w3[ft]^T xp), bf16."""
                XP, HT = state["XP"], state["HT"]
                W1, W3 = state.pop(("W13", ft))
                p1 = psum.tile([128, 1024], f32, tag="ps", name=f"p1_{ft}")
                p3 = psum.tile([128, 1024], f32, tag="ps", name=f"p3_{ft}")
                for dt in range(NDT):
                    st, sp = dt == 0, dt == NDT - 1
                    for (o, do, n) in chunks:
                        nc.tensor.matmul(p1[:, o:o + n], W1[:, dt, :],
                                         XP[:, dt, do:do + n],
                                         start=st, stop=sp)
                    for (o, do, n) in chunks:
                        nc.tensor.matmul(p3[:, o:o + n], W3[:, dt, :],
                                         XP[:, dt, do:do + n],
                                         start=st, stop=sp)
                for (o, do, n) in chunks:
                    sa = spool.tile([128, 512], f32, tag="silu",
                                    name=f"sa{ft}_{o}")
                    nc.scalar.activation(sa[:, :n], p1[:, o:o + n], Act.Silu)
                    nc.vector.tensor_mul(HT[:, ft, do:do + n], sa[:, :n],
                                         p3[:, o:o + n])

            def mm2(dt):
                HT = state["HT"]
                W2 = state.pop(("W2", dt))
                yo = outs.tile([128, cap], bf, tag="yo", name=f"yo{dt}")
                py = psum.tile([128, 1024], f32, tag="ps", name=f"py{dt}")
                for ft in range(NFT):
                    st, sp = ft == 0, ft == NFT - 1
                    for (o, do, n) in chunks:
                        nc.tensor.matmul(py[:, o:o + n], W2[:, ft, :],
                                         HT[:, ft, do:do + n],
                                         start=st, stop=sp)
                for (o, do, n) in chunks:
                    nc.scalar.activation(yo[:, do:do + n], py[:, o:o + n],
                                         Act.Copy)
                nc.sync.dma_start(out=yr[:, dt, :], in_=yo)

            def shared_chunk(ch):
                W1S, W3S, W2S = state["W1S"], state["W3S"], state["W2S"]
                XF = state.pop(("XF", ch))
                HS = hts.tile([128, FS // 128, 512], bf, tag="hs",
                              name=f"hs{ch}")
                for ft in range(FS // 128):
                    p1 = psum.tile([128, 1024], f32, tag="ps",
                                   name=f"sp1_{ch}_{ft}")
                    p3 = psum.tile([128, 1024], f32, tag="ps",
                                   name=f"sp3_{ch}_{ft}")
                    for dt in range(NDT):
                        st, sp = dt == 0, dt == NDT - 1
                        nc.tensor.matmul(p1[:, 0:512],
                                         W1S[:, dt, ft * 128:(ft + 1) * 128],
                                         XF[:, dt, :], start=st, stop=sp)
                        nc.tensor.matmul(p3[:, 0:512],
                                         W3S[:, dt, ft * 128:(ft + 1) * 128],
                                         XF[:, dt, :], start=st, stop=sp)
                    sa = spool.tile([128, 512], f32, tag="silu",
                                    name=f"ssa{ch}_{ft}")
                    nc.scalar.activation(sa, p1[:, 0:512], Act.Silu)
                    nc.vector.tensor_mul(HS[:, ft, :], sa, p3[:, 0:512])
                so = outs.tile([128, NDT, 512], bf, tag="so", name=f"so{ch}")
                for dt in range(NDT):
                    py = psum.tile([128, 1024], f32, tag="ps",
                                   name=f"spy{ch}_{dt}")
                    for ft in range(FS // 128):
                        nc.tensor.matmul(py[:, 0:512],
                                         W2S[:, ft, dt * 128:(dt + 1) * 128],
                                         HS[:, ft, :],
                                         start=(ft == 0), stop=(ft == 3))
                    nc.scalar.activation(so[:, dt, :], py[:, 0:512], Act.Copy)
                    nc.sync.dma_start(
                        out=ys[:, dt, ch * 512:(ch + 1) * 512],
                        in_=so[:, dt, :])

            # ---- prologue: DMAs in consumption order -------------------
            # interleave per-dt slices of W13(0) and XP so the very first
            # ffn13 matmul only waits for ~0.3MB of DMA.
            XP = acts.tile([128, NDT, cap], bf)
            W1_0 = wst.tile([128, NDT, 128], bf, tag="w1", name="w1_0")
            W3_0 = wst.tile([128, NDT, 128], bf, tag="w3", name="w3_0")
            for dt in range(NDT):
                nc.sync.dma_start(out=W1_0[:, dt, :], in_=w1[0, :, dt, :])
                nc.sync.dma_start(out=W3_0[:, dt, :], in_=w3[0, :, dt, :])
                nc.sync.dma_start(out=XP[:, dt, :], in_=xp[:, dt, :])
            state[("W13", 0)] = (W1_0, W3_0)
            HT = htp.tile([128, NFT, cap], bf, tag="ht")
            state.update(XP=XP, HT=HT)
            for ft in range(1, 6):
                load_w13(ft)

            def load_shared_const(which):
                # issued mid-loop so the 4MB burst doesn't starve the
                # w13 stream during the first f-tiles
                if which == 0:
                    W1S = const.tile([128, NDT, FS], bf)
                    nc.sync.dma_start(out=W1S, in_=w1s)
                    state["W1S"] = W1S
                elif which == 1:
                    W3S = const.tile([128, NDT, FS], bf)
                    nc.sync.dma_start(out=W3S, in_=w3s)
                    state["W3S"] = W3S
                else:
                    W2S = const.tile([128, FS // 128, D], bf)
                    nc.sync.dma_start(out=W2S, in_=w2s)
                    state["W2S"] = W2S

            # ---- main schedule ----------------------------------------
            # routed f-tiles with shared chunks 0/1 interleaved; shared
            # chunks 2/3 cover the HT-finalization latency before mm2;
            # mm2 runs last so the kernel tail is one small yr DMA.
            for ft in range(NFT):
                if ft + 6 < NFT:
                    load_w13(ft + 6)
                elif ft + 6 < NFT + 4:        # prefetch first w2 dt-slabs
                    load_w2(ft + 6 - NFT)
                ffn13(ft)
                if ft in (7, 9, 11):
                    load_shared_const((ft - 7) // 2)
                if ft in (8, 16, 24, 27):
                    load_xf({8: 0, 16: 1, 24: 2, 27: 3}[ft])
                if ft in (14, 22):
                    shared_chunk({14: 0, 22: 1}[ft])
            shared_chunk(2)
            shared_chunk(3)
            for dt in range(NDT):
                mm2(dt)
                if dt + 4 < NDT:
                    load_w2(dt + 4)

    nc.compile()
    return nc


def _get_program(cap):
    if cap not in _NC:
        _NC[cap] = _build_program(cap)
    return _NC[cap]


def _pack_dff_tiles(w):
    """[D, Fx] -> [Fx/128, 128, D/128, 128] tiles: t[ft, p, a, f]."""
    d, fx = w.shape
    return np.ascontiguousarray(
        w.reshape(d // 128, 128, fx // 128, 128).transpose(2, 1, 0, 3))


def _pack_part(w):
    """[D, N] -> [128, D/128, N]: t[p, a, n] = w[a*128+p, n]."""
    d, n = w.shape
    return np.ascontiguousarray(w.reshape(d // 128, 128, n).transpose(1, 0, 2))


def _unpack_part(t):
    """[128, A, N] -> [A*128, N]."""
    p, a, n = t.shape
    return t.transpose(1, 0, 2).reshape(a * p, n)


def kernel(hidden_states, gate_W, w1_e, w3_e, w2_e, w1_s, w3_s, w2_s):
    global LAST_RESULT
    x = np.ascontiguousarray(
        np.asarray(hidden_states, np.float32).reshape(T, D))

    # ---- host routing + gate values ---------------------------------
    gate_W = np.asarray(gate_W, np.float32)
    logits = x @ gate_W.T                       # [T, E]
    m = logits.max(axis=1, keepdims=True)
    p = np.exp(logits - m)
    probs = p / p.sum(axis=1, keepdims=True)
    order = np.argsort(-probs, axis=1, kind="stable")[:, :K]   # [T, K]

    idx = []            # token indices routed to each expert
    for e in range(E):
        te = np.where((order == e).any(axis=1))[0]
        idx.append(te)
    cap = max(544, -(-max(len(te) for te in idx) // 32) * 32)

    # ---- build device inputs ----------------------------------------
    xT = np.ascontiguousarray(x.T)              # [D, T] fp32
    xf_bf = _pack_part(xT.astype(BF16))         # [128, 8, T]

    w1_e = np.asarray(w1_e, np.float32)
    w3_e = np.asarray(w3_e, np.float32)
    w2_e = np.asarray(w2_e, np.float32)
    w1_s = np.asarray(w1_s, np.float32)
    w3_s = np.asarray(w3_s, np.float32)
    # fold alpha/NS (an exact power of two) into the shared down-proj
    w2_s = np.asarray(w2_s, np.float32) * (ALPHA / NS)

    nc = _get_program(cap)
    in_maps = []
    for c in range(N_CORES):
        te = idx[c]
        xp = np.zeros((D, cap), dtype=BF16)
        xp[:, :len(te)] = xT[:, te].astype(BF16)
        in_maps.append({
            "xp": _pack_part(xp),
            "xf": xf_bf,
            "w1": _pack_dff_tiles(w1_e[c].astype(BF16)),
            "w3": _pack_dff_tiles(w3_e[c].astype(BF16)),
            "w2": _pack_dff_tiles(w2_e[c].astype(BF16)),
            "w1s": _pack_part(w1_s[c].astype(BF16)),
            "w3s": _pack_part(w3_s[c].astype(BF16)),
            "w2s": _pack_part(w2_s[c].astype(BF16)),
        })

    res = run_bass_kernel_spmd(nc, in_maps, list(range(N_CORES)))
    LAST_RESULT = res

    # ---- host combine: gate-weighted scatter of yr + sum of ys ------
    outT = np.zeros((D, T), np.float32)
    for c in range(N_CORES):
        outT += _unpack_part(res.results[c]["ys"].astype(np.float32))
        te = idx[c]
        yrc = _unpack_part(res.results[c]["yr"].astype(np.float32))
        gate = (1.0 - ALPHA) * probs[te, c]
        outT[:, te] += yrc[:, :len(te)] * gate[None, :]

    return np.ascontiguousarray(outT.T).reshape(B, S, D).astype(np.float32)
